# revision 1
# baseline (speedup 1.0000x reference)
"""GraphTransformer (2x PyG TransformerConv + linear) on 8 trn2 NeuronCores.

Strategy: edges sorted by destination, packed into fixed-size blocks
(512 edge slots / 64 dst slots, segments never split). Contiguous dst
ranges are sharded across the 8 cores (edge-balanced). Three SPMD
launches:
  P : per-core slice projections  kv1 = x@[Wk1|Wv1],  qs1 = x@[Wq1|Ws1]
  L1: per-edge gather of kv1 rows (indirect DMA), attention via PE
      matmuls against per-block one-hot segment matrices M, softmax
      without max-subtraction (scores are O(1)), segment sums via
      M^T @ [weighted_v || exp], dense normalize+skip+relu pass, then
      the layer-2 projection kvqs2 = h1@[Wk2|Wv2|Wq2|Ws2]
  L2: same loop on kv2 (single head), final linear to [N, 2]
Host does only index bookkeeping, concatenation and transposes.
"""
import sys

sys.path.insert(0, "/opt/trn_rl_repo")
import numpy as np
import concourse.bass as bass
import concourse.bacc as bacc
import concourse.tile as tile
from concourse import mybir
from concourse.bass_utils import run_bass_kernel_spmd
from concourse.masks import make_identity

F32 = mybir.dt.float32
I32 = mybir.dt.int32
NCORES = 8
NNODE = 50000
EPB, DPB, TSUB = 512, 64, 4          # edges/block, dst slots/block, 128-edge subtiles
NOWN = 6400                          # padded own-node slots per core (50*128)
NTIL = NOWN // 128

_built = {}


def _bc(ap, p):
    """Broadcast a [1, n] DRAM AP across p partitions."""
    return bass.AP(tensor=ap.tensor, offset=ap.offset, ap=[[0, p]] + list(ap.ap[1:]))


def _build_P():
    nc = bacc.Bacc()
    xT = nc.declare_dram_parameter("xT", [64, NOWN], F32, isOutput=False)
    W1 = nc.declare_dram_parameter("W1", [64, 1024], F32, isOutput=False)
    b1 = nc.declare_dram_parameter("b1", [1, 1024], F32, isOutput=False)
    kv = nc.declare_dram_parameter("kv", [NOWN, 512], F32, isOutput=True)
    qs = nc.declare_dram_parameter("qs", [NOWN, 512], F32, isOutput=True)
    with tile.TileContext(nc) as tc:
        with tc.tile_pool(name="one", bufs=1) as one, \
             tc.tile_pool(name="sb", bufs=3) as sb, \
             tc.tile_pool(name="ps", bufs=2, space="PSUM") as ps:
            W1t = one.tile([64, 1024], F32)
            nc.sync.dma_start(out=W1t[:], in_=W1[:])
            b1t = one.tile([128, 1024], F32)
            nc.sync.dma_start(out=b1t[:], in_=_bc(b1[:], 128))
            for i in range(NTIL):
                r = slice(i * 128, (i + 1) * 128)
                xt = sb.tile([64, 128], F32, tag="xt")
                nc.sync.dma_start(out=xt[:], in_=xT[:, r])
                o_kv = sb.tile([128, 512], F32, tag="okv")
                o_qs = sb.tile([128, 512], F32, tag="oqs")
                for j, od in ((0, o_kv), (1, o_qs)):
                    pp = ps.tile([128, 512], F32, tag=f"p{j}")
                    nc.tensor.matmul(out=pp[:], lhsT=xt[:],
                                     rhs=W1t[:, j * 512:(j + 1) * 512],
                                     start=True, stop=True)
                    nc.vector.tensor_add(od[:], pp[:], b1t[:, j * 512:(j + 1) * 512])
                nc.sync.dma_start(out=kv[r, :], in_=o_kv[:])
                nc.sync.dma_start(out=qs[r, :], in_=o_qs[:])
    nc.finalize()
    return nc


def _build_conv(B, DKV, H, OUTW):
    """Gather/attention launch. DKV: gathered row width (k|v), H heads of 64.
    OUTW: trailing dense-output width (256 for L1's kvqs2, 2 for L2's final)."""
    DH = DKV // 2                     # features per head-group (k or v part)
    SW = DKV // 2 + H                 # stage row: msg(DH) + per-head sums(H)
    nc = bacc.Bacc()
    kvf = nc.declare_dram_parameter("kvf", [NNODE, DKV], F32, isOutput=False)
    qtab = nc.declare_dram_parameter("qtab", [NOWN + 1, DH], F32, isOutput=False)
    sktab = nc.declare_dram_parameter("sktab", [NOWN, DH], F32, isOutput=False)
    srcb = nc.declare_dram_parameter("srcb", [B, 128, TSUB], I32, isOutput=False)
    Mb = nc.declare_dram_parameter("Mb", [B, 128, TSUB * DPB], F32, isOutput=False)
    qrow = nc.declare_dram_parameter("qrow", [B, DPB, 1], I32, isOutput=False)
    strow = nc.declare_dram_parameter("strow", [NOWN, 1], I32, isOutput=False)
    WO = nc.declare_dram_parameter("WO", [DH, OUTW], F32, isOutput=False)
    bO = nc.declare_dram_parameter("bO", [1, OUTW], F32, isOutput=False)
    outt = nc.declare_dram_parameter("outt", [NOWN, OUTW], F32, isOutput=True)
    stage = nc.dram_tensor("stage", [B * DPB, SW], F32)

    with tile.TileContext(nc) as tc:
        with tc.tile_pool(name="one", bufs=1) as one:
            ident = one.tile([128, 128], F32)
            make_identity(nc, ident[:])
            nwo = (DH + 127) // 128
            WOt = [one.tile([min(128, DH - 128 * k), OUTW], F32, tag=f"wo{k}",
                            name=f"wo{k}")
                   for k in range(nwo)]
            for k in range(nwo):
                nc.sync.dma_start(out=WOt[k][:], in_=WO[128 * k: 128 * k + WOt[k].shape[0], :])
            bOt = one.tile([128, OUTW], F32)
            nc.sync.dma_start(out=bOt[:], in_=_bc(bO[:], 128))

            # ---- block loop: gather + attention partial sums ----
            with tc.tile_pool(name="sb", bufs=3) as sb, \
                 tc.tile_pool(name="ps", bufs=2, space="PSUM") as ps:
                for b in range(B):
                    src_t = sb.tile([128, TSUB], I32, tag="src")
                    nc.sync.dma_start(out=src_t[:], in_=srcb[b])
                    qr_t = sb.tile([DPB, 1], I32, tag="qr")
                    nc.sync.dma_start(out=qr_t[:], in_=qrow[b])
                    M_t = sb.tile([128, TSUB * DPB], F32, tag="M")
                    nc.sync.dma_start(out=M_t[:], in_=Mb[b])
                    qrows = sb.tile([DPB, DH], F32, tag="qrows")
                    nc.gpsimd.indirect_dma_start(
                        out=qrows[:], out_offset=None, in_=qtab[:],
                        in_offset=bass.IndirectOffsetOnAxis(ap=qr_t[:, :1], axis=0))
                    vwe = sb.tile([128, TSUB, DH + H], F32, tag="vwe")
                    agg = ps.tile([DPB, SW], F32, tag="agg")
                    for t in range(TSUB):
                        kvt = sb.tile([128, DKV], F32, tag=f"kv{t}")
                        nc.gpsimd.indirect_dma_start(
                            out=kvt[:], out_offset=None, in_=kvf[:],
                            in_offset=bass.IndirectOffsetOnAxis(
                                ap=src_t[:, t:t + 1], axis=0))
                        Mcol = M_t[:, t * DPB:(t + 1) * DPB]
                        mtp = ps.tile([DPB, 128], F32, tag="mt")
                        nc.tensor.transpose(out=mtp[:], in_=Mcol, identity=ident[:])
                        mts = sb.tile([DPB, 128], F32, tag="mts")
                        nc.vector.tensor_copy(mts[:], mtp[:])
                        qep = ps.tile([128, DH], F32, tag="qe")
                        nc.tensor.matmul(out=qep[:], lhsT=mts[:], rhs=qrows[:],
                                         start=True, stop=True)
                        prod = sb.tile([128, DH], F32, tag="prod")
                        nc.vector.tensor_mul(prod[:], qep[:], kvt[:, 0:DH])
                        alpha = sb.tile([128, H], F32, tag="alpha")
                        nc.vector.reduce_sum(
                            out=alpha[:],
                            in_=prod[:].rearrange("p (h d) -> p h d", h=H),
                            axis=mybir.AxisListType.X)
                        expv = vwe[:, t, DH:DH + H]
                        nc.scalar.activation(expv, alpha[:],
                                             mybir.ActivationFunctionType.Exp,
                                             scale=0.125)
                        nc.vector.tensor_mul(
                            vwe[:, t, 0:DH].rearrange("p (h d) -> p h d", h=H),
                            kvt[:, DH:DKV].rearrange("p (h d) -> p h d", h=H),
                            expv.unsqueeze(2).to_broadcast([128, H, 64]))
                        nc.tensor.matmul(out=agg[:], lhsT=Mcol, rhs=vwe[:, t, :],
                                         start=(t == 0), stop=(t == TSUB - 1))
                    aggs = sb.tile([DPB, SW], F32, tag="aggs")
                    nc.vector.tensor_copy(aggs[:], agg[:])
                    nc.sync.dma_start(out=stage[b * DPB:(b + 1) * DPB, :], in_=aggs[:])

            # ---- dense pass: normalize + skip + relu + output matmul ----
            with tc.tile_pool(name="sb2", bufs=3) as sb, \
                 tc.tile_pool(name="ps2", bufs=2, space="PSUM") as ps:
                for i in range(NTIL):
                    r = slice(i * 128, (i + 1) * 128)
                    st_t = sb.tile([128, 1], I32, tag="st")
                    nc.sync.dma_start(out=st_t[:], in_=strow[r])
                    pre = sb.tile([128, SW], F32, tag="pre")
                    nc.gpsimd.indirect_dma_start(
                        out=pre[:], out_offset=None, in_=stage[:],
                        in_offset=bass.IndirectOffsetOnAxis(ap=st_t[:, :1], axis=0))
                    sc = sb.tile([128, H], F32, tag="sc")
                    nc.vector.tensor_scalar_max(sc[:], pre[:, DH:SW], 1e-30)
                    rs = sb.tile([128, H], F32, tag="rs")
                    nc.vector.reciprocal(rs[:], sc[:])
                    sk = sb.tile([128, DH], F32, tag="sk")
                    nc.sync.dma_start(out=sk[:], in_=sktab[r, :])
                    h = sb.tile([128, DH], F32, tag="h")
                    nc.vector.tensor_mul(
                        h[:].rearrange("p (g d) -> p g d", g=H),
                        pre[:, 0:DH].rearrange("p (g d) -> p g d", g=H),
                        rs[:].unsqueeze(2).to_broadcast([128, H, 64]))
                    nc.vector.tensor_add(h[:], h[:], sk[:])
                    nc.scalar.activation(h[:], h[:],
                                         mybir.ActivationFunctionType.Relu)
                    op = ps.tile([128, OUTW], F32, tag="op")
                    for k in range(nwo):
                        kw = WOt[k].shape[0]
                        tp = ps.tile([kw, 128], F32, tag="tp")
                        nc.tensor.transpose(out=tp[:], in_=h[:, 128 * k:128 * k + kw],
                                            identity=ident[:])
                        ts_ = sb.tile([kw, 128], F32, tag="ts")
                        nc.vector.tensor_copy(ts_[:], tp[:])
                        nc.tensor.matmul(out=op[:], lhsT=ts_[:], rhs=WOt[k][:],
                                         start=(k == 0), stop=(k == nwo - 1))
                    oo = sb.tile([128, OUTW], F32, tag="oo")
                    nc.vector.tensor_add(oo[:], op[:], bOt[:])
                    nc.sync.dma_start(out=outt[r, :], in_=oo[:])
    nc.finalize()
    return nc


def _prep(edge_index):
    """Sort/pack the graph. Returns per-core block metadata."""
    src = np.ascontiguousarray(edge_index[0]).astype(np.int64)
    dst = np.ascontiguousarray(edge_index[1]).astype(np.int64)
    E = src.shape[0]
    order = np.argsort(dst, kind="stable")
    s_sorted = src[order].astype(np.int32)
    d_sorted = dst[order]
    deg = np.bincount(d_sorted, minlength=NNODE)
    cume = np.concatenate([[0], np.cumsum(deg)])          # edge start per node
    # core boundaries: balanced edge counts at node granularity
    targets = [round(E * c / NCORES) for c in range(1, NCORES)]
    nb = [0] + [int(np.searchsorted(cume, t)) for t in targets] + [NNODE]
    cores = []
    for c in range(NCORES):
        n0, n1 = nb[c], nb[c + 1]
        assert n1 - n0 <= NOWN, (c, n1 - n0)
        blocks = []   # list of (list of (node, edge_lo, edge_hi))
        cur, ecnt = [], 0
        for n in range(n0, n1):
            g = int(deg[n])
            assert g <= EPB
            if len(cur) >= DPB or ecnt + g > EPB:
                blocks.append(cur)
                cur, ecnt = [], 0
            cur.append(n)
            ecnt += g
        if cur:
            blocks.append(cur)
        cores.append((n0, n1, blocks))
    B = max(len(cb) for _, _, cb in cores)
    per_core = []
    for c in range(NCORES):
        n0, n1, blocks = cores[c]
        srcb = np.zeros((B, EPB), np.int32)
        Mb = np.zeros((B, 128, TSUB * DPB), np.float32)
        qrow = np.full((B, DPB, 1), NOWN, np.int32)
        strow = np.zeros((NOWN, 1), np.int32)
        for b, nodes in enumerate(blocks):
            e = 0
            for slot, n in enumerate(nodes):
                qrow[b, slot, 0] = n - n0
                strow[n - n0, 0] = b * DPB + slot
                lo, hi = cume[n], cume[n + 1]
                g = hi - lo
                srcb[b, e:e + g] = s_sorted[lo:hi]
                for k in range(g):
                    ee = e + k
                    Mb[b, ee % 128, (ee // 128) * DPB + slot] = 1.0
                e += g
        # edge slot e -> subtile e//128, partition e%128
        srcb = srcb.reshape(B, TSUB, 128).transpose(0, 2, 1).copy()
        per_core.append(dict(n0=n0, n1=n1, srcb=srcb, Mb=Mb, qrow=qrow,
                             strow=strow))
    return B, per_core


def kernel(x, edge_index, Wq1, bq1, Wk1, bk1, Wv1, bv1, Ws1, bs1,
           Wq2, bq2, Wk2, bk2, Wv2, bv2, Ws2, bs2, Wl, bl):
    x = np.asarray(x, np.float32)
    B, per_core = _prep(np.asarray(edge_index))

    if "P" not in _built:
        _built["P"] = _build_P()
    if ("L1", B) not in _built:
        _built[("L1", B)] = _build_conv(B, 512, 4, 256)
    if ("L2", B) not in _built:
        _built[("L2", B)] = _build_conv(B, 128, 1, 2)

    W1 = np.concatenate([Wk1, Wv1, Wq1, Ws1], axis=1).astype(np.float32)
    b1 = np.concatenate([bk1, bv1, bq1, bs1])[None, :].astype(np.float32)
    W2 = np.concatenate([Wk2, Wv2, Wq2, Ws2], axis=1).astype(np.float32)
    b2 = np.concatenate([bk2, bv2, bq2, bs2])[None, :].astype(np.float32)
    cids = list(range(NCORES))

    # ---- launch P: projections of own slices ----
    xTs = []
    for pc in per_core:
        xs = np.zeros((NOWN, 64), np.float32)
        xs[: pc["n1"] - pc["n0"]] = x[pc["n0"]: pc["n1"]]
        xTs.append(np.ascontiguousarray(xs.T))
    resP = run_bass_kernel_spmd(
        _built["P"],
        [{"xT": xTs[c], "W1": W1, "b1": b1} for c in cids], cids)
    tP = resP.exec_time_ns

    kv1 = np.concatenate(
        [resP.results[c]["kv"][: per_core[c]["n1"] - per_core[c]["n0"]]
         for c in cids], axis=0)                       # [N, 512]
    # ---- launch L1 ----
    in1 = []
    for c in cids:
        pc = per_core[c]
        qs = resP.results[c]["qs"]                     # [NOWN, 512] q|sk
        qtab = np.zeros((NOWN + 1, 256), np.float32)
        qtab[:NOWN] = qs[:, :256]
        in1.append(dict(kvf=kv1, qtab=qtab, sktab=np.ascontiguousarray(qs[:, 256:]),
                        srcb=pc["srcb"], Mb=pc["Mb"], qrow=pc["qrow"],
                        strow=pc["strow"], WO=W2, bO=b2))
    res1 = run_bass_kernel_spmd(_built[("L1", B)], in1, cids)
    t1 = res1.exec_time_ns

    kv2 = np.concatenate(
        [res1.results[c]["outt"][: per_core[c]["n1"] - per_core[c]["n0"], :128]
         for c in cids], axis=0)                       # [N, 128]
    # ---- launch L2 ----
    Wlc = np.asarray(Wl, np.float32)
    blc = np.asarray(bl, np.float32)[None, :]
    in2 = []
    for c in cids:
        pc = per_core[c]
        o1 = res1.results[c]["outt"]                   # [NOWN, 256] k2|v2|q2|sk2
        qtab2 = np.zeros((NOWN + 1, 64), np.float32)
        qtab2[:NOWN] = o1[:, 128:192]
        in2.append(dict(kvf=kv2, qtab=qtab2, sktab=np.ascontiguousarray(o1[:, 192:]),
                        srcb=pc["srcb"], Mb=pc["Mb"], qrow=pc["qrow"],
                        strow=pc["strow"], WO=Wlc, bO=blc))
    res2 = run_bass_kernel_spmd(_built[("L2", B)], in2, cids)
    t2 = res2.exec_time_ns

    out = np.concatenate(
        [res2.results[c]["outt"][: per_core[c]["n1"] - per_core[c]["n0"]]
         for c in cids], axis=0)
    kernel.exec_times = (tP, t1, t2)
    return out



# revision 2
# speedup vs baseline: 1.0554x; 1.0554x over previous
"""GraphTransformer (2x PyG TransformerConv + linear) on 8 trn2 NeuronCores.

v4: 1024-edge/64-slot units, dma_gather batched gathers (<=1024 idx, i16
    with host-baked lo/hi range splits), fused DVE ops, no scatters.
  P : projections (node space): kvtab [NOWN,512] bf16 = [k(256)|v(256)],
      qtab/sktab [NOWN+1,256] bf16. q pre-scaled by 1/8. Bias via x-aug.
  Lx: per unit: one meta DMA (wrapped i16 gather indices + slot labels),
      2-range kv dma_gather + q dma_gather. Per 128-edge subtile: alpha_h
      via scalar_tensor_tensor accum, exp on Act, M'_h = (iota==slot)*expv_h
      via tensor_scalar, PE matmuls accumulate msg into agg[:,h,0:64] and
      denominators (rhs = const ones col) into agg[:,h,64:65] in one psum
      group per head-bank. Act copy + one stage write per unit. Dense pass
      per 512 slots: stage read, sk (gathered for L1 / direct for L2),
      normalize+skip (stt), relu (Act), transpose+WO+bias -> outt.
  L1 outt [SLOC,256] = [k2|v2|q2|sk2]; L2 outt [SLOC,2] final.
"""
import sys

sys.path.insert(0, "/opt/trn_rl_repo")
import numpy as np
import ml_dtypes
import concourse.bass as bass
import concourse.bacc as bacc
import concourse.tile as tile
from concourse import mybir
from concourse.bass_utils import run_bass_kernel_spmd
from concourse.masks import make_identity

F32 = mybir.dt.float32
BF16 = mybir.dt.bfloat16
I32 = mybir.dt.int32
I16 = mybir.dt.int16
BF = ml_dtypes.bfloat16

NCORES = 8
NNODE = 50000
UE, DPB, TSUB = 1024, 64, 8        # edges/unit, slots/unit, subtiles/unit
NOWN = 6400
NTIL = NOWN // 128
GP = 10
RSPLIT = 32768

_built = {}


def _build_P():
    nc = bacc.Bacc()
    xT = nc.declare_dram_parameter("xT", [65, NOWN], BF16, isOutput=False)
    W1 = nc.declare_dram_parameter("W1", [65, 1024], BF16, isOutput=False)
    kvt_o = nc.declare_dram_parameter("kvt", [NOWN, 512], BF16, isOutput=True)
    qtab = nc.declare_dram_parameter("qtab", [NOWN + 1, 256], BF16, isOutput=True)
    sktab = nc.declare_dram_parameter("sktab", [NOWN + 1, 256], BF16,
                                      isOutput=True)
    with tile.TileContext(nc) as tc:
        with tc.tile_pool(name="one", bufs=1) as one, \
             tc.tile_pool(name="sc", bufs=2) as scp, \
             tc.tile_pool(name="ps", bufs=2, space="PSUM") as ps:
            xt = one.tile([65, NOWN], BF16)
            nc.sync.dma_start(out=xt[:], in_=xT[:])
            W1t = one.tile([65, 1024], BF16)
            nc.sync.dma_start(out=W1t[:], in_=W1[:])
            zrow = one.tile([1, 256], BF16)
            nc.vector.memset(zrow[:], 0.0)
            nc.sync.dma_start(out=qtab[NOWN:NOWN + 1, :], in_=zrow[:])
            nc.sync.dma_start(out=sktab[NOWN:NOWN + 1, :], in_=zrow[:])
            for g in range(NTIL // GP):
                qbig = scp.tile([128, GP, 256], BF16, tag="qbig")
                sbig = scp.tile([128, GP, 256], BF16, tag="sbig")
                kbig = scp.tile([128, GP, 512], BF16, tag="kbig")
                for j in range(GP):
                    i = g * GP + j
                    r = slice(i * 128, (i + 1) * 128)
                    p0 = ps.tile([128, 512], F32, tag="p0")
                    nc.tensor.matmul(out=p0[:], lhsT=xt[:, r], rhs=W1t[:, 0:512],
                                     start=True, stop=True)
                    p1 = ps.tile([128, 512], F32, tag="p1")
                    nc.tensor.matmul(out=p1[:], lhsT=xt[:, r],
                                     rhs=W1t[:, 512:1024], start=True, stop=True)
                    nc.vector.tensor_copy(kbig[:, j, :], p0[:])
                    nc.scalar.activation(qbig[:, j, :], p1[:, 0:256],
                                         mybir.ActivationFunctionType.Copy)
                    nc.scalar.activation(sbig[:, j, :], p1[:, 256:512],
                                         mybir.ActivationFunctionType.Copy)
                rows = slice(g * GP * 128, (g + 1) * GP * 128)
                nc.sync.dma_start(
                    out=kvt_o[rows, :].rearrange("(j p) w -> p j w", p=128),
                    in_=kbig[:])
                nc.sync.dma_start(
                    out=qtab[rows, :].rearrange("(j p) w -> p j w", p=128),
                    in_=qbig[:])
                nc.sync.dma_start(
                    out=sktab[rows, :].rearrange("(j p) w -> p j w", p=128),
                    in_=sbig[:])
    nc.finalize()
    return nc


def _build_conv(NU, NTAB, H, OUTW, QR, skdirect, NLO, hsplit):
    """H heads of 64. kv row = [k(64H)|v(64H)]. NLO: fixed lo-range idx
    count per unit (mult of 128); hi = UE-NLO. hsplit: M' heads 0..hsplit-1
    on DVE, rest on Pool."""
    DH = 64 * H
    QW = max(DH, 128)              # q-table row (>=256B for dma_gather)
    KW = 128 * H
    SW = 65 * H
    SKW = DH
    SLOC = NU * DPB
    SB = SLOC // 512
    nc = bacc.Bacc(dynamic_dma_scratch_size=1 << 17)
    kvf = nc.declare_dram_parameter("kvf", [NTAB, KW], BF16, isOutput=False)
    qtab = nc.declare_dram_parameter("qtab", [QR, QW], BF16, isOutput=False)
    sktab = nc.declare_dram_parameter("sktab", [QR, SKW], BF16, isOutput=False)
    midx = nc.declare_dram_parameter("midx", [NU, 128, 136], I16, isOutput=False)
    WO = nc.declare_dram_parameter("WO", [SKW + 1, OUTW], BF16, isOutput=False)
    outt = nc.declare_dram_parameter("outt", [SLOC, OUTW], BF16, isOutput=True)
    stage = nc.dram_tensor("stage", [SLOC, SW], BF16)
    if not skdirect:
        stn = nc.declare_dram_parameter("stn", [SB, 128, 32], I16, isOutput=False)

    with tile.TileContext(nc) as tc:
        with tc.tile_pool(name="one", bufs=1) as one:
            iota = one.tile([128, 64], I16)
            nc.gpsimd.iota(iota[:], pattern=[[1, 64]], base=0,
                           channel_multiplier=0)
            onesrow = one.tile([1, 128], BF16)
            nc.vector.memset(onesrow[:], 1.0)
            onescol = one.tile([128, 1], BF16)
            nc.vector.memset(onescol[:], 1.0)
            ident = one.tile([128, 128], BF16)
            make_identity(nc, ident[:])
            nkw = max(1, SKW // 128)
            kww = SKW // nkw
            WOt = [one.tile([kww, OUTW], BF16, name=f"wo{k}")
                   for k in range(nkw)]
            for k in range(nkw):
                nc.sync.dma_start(out=WOt[k][:], in_=WO[k*kww:(k+1)*kww, :])
            WOb = one.tile([1, OUTW], BF16)
            nc.sync.dma_start(out=WOb[:], in_=WO[SKW:SKW+1, :])

            with tc.tile_pool(name="sb", bufs=3) as sb, \
                 tc.tile_pool(name="sm", bufs=3) as sm, \
                 tc.tile_pool(name="ps", bufs=2, space="PSUM") as ps:
                nlo, nhi = NLO, UE - NLO
                for u in range(NU):
                    git = sb.tile([128, 136], I16, tag="git")
                    nc.sync.dma_start(out=git[:], in_=midx[u])
                    slt = sb.tile([128, 8], F32, tag="slt")
                    nc.gpsimd.tensor_copy(slt[:], git[:, 128:136])
                    kvg = sb.tile([128, TSUB, KW], BF16, tag="kvg")
                    nc.gpsimd.dma_gather(
                        out_ap=kvg[:, 0:nlo // 128, :], in_ap=kvf[0:RSPLIT, :],
                        idxs_ap=git[:, 0:nlo // 16], num_idxs=nlo,
                        num_idxs_reg=nlo, elem_size=KW)
                    nc.gpsimd.dma_gather(
                        out_ap=kvg[:, nlo // 128:TSUB, :],
                        in_ap=kvf[RSPLIT:NTAB, :],
                        idxs_ap=git[:, nlo // 16:64],
                        num_idxs=nhi, num_idxs_reg=nhi, elem_size=KW)
                    qg = sb.tile([128, TSUB, QW], BF16, tag="qg")
                    nc.gpsimd.dma_gather(
                        out_ap=qg[:], in_ap=qtab[:], idxs_ap=git[:, 64:128],
                        num_idxs=UE, num_idxs_reg=UE, elem_size=QW)
                    agg = ps.tile([64, H, 512], F32, tag="agg")
                    for t in range(TSUB):
                        alpha = sm.tile([128, H], F32, tag=f"al{t % 2}")
                        sc = sm.tile([128, H, 64], BF16, tag=f"sc{t % 2}")
                        for h in range(H):
                            nc.vector.scalar_tensor_tensor(
                                out=sc[:, h, :], in0=qg[:, t, 64*h:64*h+64],
                                scalar=1.0, in1=kvg[:, t, 64*h:64*h+64],
                                op0=mybir.AluOpType.mult,
                                op1=mybir.AluOpType.mult,
                                accum_out=alpha[:, h:h+1])
                        expv = sm.tile([128, H], F32, tag=f"ex{t % 2}")
                        nc.scalar.activation(expv[:], alpha[:],
                                             mybir.ActivationFunctionType.Exp)
                        mp = sm.tile([128, H, 64], BF16, tag=f"mp{t % 2}")
                        for h in range(H):
                            hv = h + 0.5 * (t % 2)
                            eng = nc.vector if hv < hsplit else nc.gpsimd
                            eng.tensor_scalar(
                                out=mp[:, h, :], in0=iota[:],
                                scalar1=slt[:, t:t+1], scalar2=expv[:, h:h+1],
                                op0=mybir.AluOpType.is_equal,
                                op1=mybir.AluOpType.mult)
                        for h in range(H):
                            nc.tensor.matmul(
                                out=agg[:, h, 0:64], lhsT=mp[:, h, :],
                                rhs=kvg[:, t, DH+64*h:DH+64*h+64],
                                start=(t == 0), stop=False,
                                skip_group_check=True)
                            nc.tensor.matmul(
                                out=agg[:, h, 64:65], lhsT=mp[:, h, :],
                                rhs=onescol[:],
                                start=False, stop=(t == TSUB - 1),
                                skip_group_check=True)
                    aggs = sm.tile([64, SW], BF16, tag="aggs")
                    nc.scalar.activation(
                        aggs[:].rearrange("p (h w) -> p h w", h=H),
                        agg[:, :, 0:65], mybir.ActivationFunctionType.Copy)
                    nc.sync.dma_start(out=stage[u * DPB:(u + 1) * DPB, :],
                                      in_=aggs[:])

            with tc.tile_pool(name="sb2", bufs=3) as sb, \
                 tc.tile_pool(name="ps2", bufs=2, space="PSUM") as ps:
                nk = max(1, SKW // 128)
                kw = SKW // nk
                for i in range(SB):
                    r0 = i * 512
                    stg = sb.tile([128, 4, SW], BF16, tag="stg")
                    nc.sync.dma_start(
                        out=stg[:],
                        in_=stage[r0:r0 + 512, :].rearrange(
                            "(c p) w -> p c w", p=128))
                    skt = sb.tile([128, 4, SKW], BF16, tag="skt")
                    if skdirect:
                        nc.sync.dma_start(
                            out=skt[:],
                            in_=sktab[r0:r0 + 512, :].rearrange(
                                "(c p) w -> p c w", p=128))
                    else:
                        sti = sb.tile([128, 32], I16, tag="sti")
                        nc.sync.dma_start(out=sti[:], in_=stn[i])
                        nc.gpsimd.dma_gather(
                            out_ap=skt[:], in_ap=sktab[:], idxs_ap=sti[:],
                            num_idxs=512, num_idxs_reg=512, elem_size=SKW)
                    den = sb.tile([128, 4, H], F32, tag="den")
                    nc.vector.tensor_scalar_max(den[:], stg[:, :, 64::65], 1e-30)
                    rs = sb.tile([128, 4, H], F32, tag="rs")
                    nc.vector.reciprocal(rs[:], den[:])
                    htl = sb.tile([128, 4, SKW], BF16, tag="htl")
                    for c in range(4):
                        for h in range(H):
                            nc.vector.scalar_tensor_tensor(
                                out=htl[:, c, 64*h:64*h+64],
                                in0=stg[:, c, 65*h:65*h+64],
                                scalar=rs[:, c, h:h+1],
                                in1=skt[:, c, 64*h:64*h+64],
                                op0=mybir.AluOpType.mult,
                                op1=mybir.AluOpType.add)
                    relu = sb.tile([128, 4, SKW], BF16, tag="relu")
                    nc.scalar.activation(relu[:], htl[:],
                                         mybir.ActivationFunctionType.Relu)
                    oo = sb.tile([128, 4, OUTW], BF16, tag="oo")
                    for c in range(4):
                        op = ps.tile([128, OUTW], F32, tag="op")
                        nc.tensor.matmul(out=op[:], lhsT=onesrow[:],
                                         rhs=WOb[:], start=True, stop=False,
                                         skip_group_check=True)
                        for k in range(nk):
                            tp = ps.tile([kw, 128], BF16, tag=f"tp{k}")
                            nc.tensor.transpose(
                                out=tp[:], in_=relu[:, c, k*kw:(k+1)*kw],
                                identity=ident[:])
                            tps = sb.tile([kw, 128], BF16, tag=f"tps{k}")
                            nc.scalar.activation(
                                tps[:], tp[:], mybir.ActivationFunctionType.Copy)
                            nc.tensor.matmul(out=op[:], lhsT=tps[:],
                                             rhs=WOt[k][:], start=False,
                                             stop=(k == nk - 1),
                                             skip_group_check=True)
                        nc.vector.tensor_copy(oo[:, c, :], op[:])
                    nc.sync.dma_start(
                        out=outt[r0:r0 + 512, :].rearrange(
                            "(c p) w -> p c w", p=128),
                        in_=oo[:])
    nc.finalize()
    return nc




# revision 3
# speedup vs baseline: 1.0651x; 1.0091x over previous
"""GraphTransformer (2x PyG TransformerConv + linear) on 8 trn2 NeuronCores.

v4: 1024-edge/64-slot units, dma_gather batched gathers (<=1024 idx, i16
    with host-baked lo/hi range splits), fused DVE ops, no scatters.
  P : projections (node space): kvtab [NOWN,512] bf16 = [k(256)|v(256)],
      qtab/sktab [NOWN+1,256] bf16. q pre-scaled by 1/8. Bias via x-aug.
  Lx: per unit: one meta DMA (wrapped i16 gather indices + slot labels),
      2-range kv dma_gather + q dma_gather. Per 128-edge subtile: alpha_h
      via scalar_tensor_tensor accum, exp on Act, M'_h = (iota==slot)*expv_h
      via tensor_scalar, PE matmuls accumulate msg into agg[:,h,0:64] and
      denominators (rhs = const ones col) into agg[:,h,64:65] in one psum
      group per head-bank. Act copy + one stage write per unit. Dense pass
      per 512 slots: stage read, sk (gathered for L1 / direct for L2),
      normalize+skip (stt), relu (Act), transpose+WO+bias -> outt.
  L1 outt [SLOC,256] = [k2|v2|q2|sk2]; L2 outt [SLOC,2] final.
"""
import sys

sys.path.insert(0, "/opt/trn_rl_repo")
import numpy as np
import ml_dtypes
import concourse.bass as bass
import concourse.bacc as bacc
import concourse.tile as tile
from concourse import mybir
from concourse.bass_utils import run_bass_kernel_spmd
from concourse.masks import make_identity

F32 = mybir.dt.float32
BF16 = mybir.dt.bfloat16
I32 = mybir.dt.int32
I16 = mybir.dt.int16
BF = ml_dtypes.bfloat16

NCORES = 8
NNODE = 50000
UE, DPB, TSUB = 1024, 64, 8        # edges/unit, slots/unit, subtiles/unit
NOWN = 6400
NTIL = NOWN // 128
GP = 10
RSPLIT = 32768

_built = {}


def _build_P():
    nc = bacc.Bacc()
    xT = nc.declare_dram_parameter("xT", [65, NOWN], BF16, isOutput=False)
    W1 = nc.declare_dram_parameter("W1", [65, 1024], BF16, isOutput=False)
    kvt_o = nc.declare_dram_parameter("kvt", [NOWN, 512], BF16, isOutput=True)
    qtab = nc.declare_dram_parameter("qtab", [NOWN + 1, 256], BF16, isOutput=True)
    sktab = nc.declare_dram_parameter("sktab", [NOWN + 1, 256], BF16,
                                      isOutput=True)
    with tile.TileContext(nc) as tc:
        with tc.tile_pool(name="one", bufs=1) as one, \
             tc.tile_pool(name="sc", bufs=2) as scp, \
             tc.tile_pool(name="ps", bufs=2, space="PSUM") as ps:
            xt = one.tile([65, NOWN], BF16)
            nc.sync.dma_start(out=xt[:], in_=xT[:])
            W1t = one.tile([65, 1024], BF16)
            nc.sync.dma_start(out=W1t[:], in_=W1[:])
            zrow = one.tile([1, 256], BF16)
            nc.vector.memset(zrow[:], 0.0)
            nc.sync.dma_start(out=qtab[NOWN:NOWN + 1, :], in_=zrow[:])
            nc.sync.dma_start(out=sktab[NOWN:NOWN + 1, :], in_=zrow[:])
            for g in range(NTIL // GP):
                qbig = scp.tile([128, GP, 256], BF16, tag="qbig")
                sbig = scp.tile([128, GP, 256], BF16, tag="sbig")
                kbig = scp.tile([128, GP, 512], BF16, tag="kbig")
                for j in range(GP):
                    i = g * GP + j
                    r = slice(i * 128, (i + 1) * 128)
                    p0 = ps.tile([128, 512], F32, tag="p0")
                    nc.tensor.matmul(out=p0[:], lhsT=xt[:, r], rhs=W1t[:, 0:512],
                                     start=True, stop=True)
                    p1 = ps.tile([128, 512], F32, tag="p1")
                    nc.tensor.matmul(out=p1[:], lhsT=xt[:, r],
                                     rhs=W1t[:, 512:1024], start=True, stop=True)
                    nc.vector.tensor_copy(kbig[:, j, :], p0[:])
                    nc.scalar.activation(qbig[:, j, :], p1[:, 0:256],
                                         mybir.ActivationFunctionType.Copy)
                    nc.scalar.activation(sbig[:, j, :], p1[:, 256:512],
                                         mybir.ActivationFunctionType.Copy)
                rows = slice(g * GP * 128, (g + 1) * GP * 128)
                nc.sync.dma_start(
                    out=kvt_o[rows, :].rearrange("(j p) w -> p j w", p=128),
                    in_=kbig[:])
                nc.sync.dma_start(
                    out=qtab[rows, :].rearrange("(j p) w -> p j w", p=128),
                    in_=qbig[:])
                nc.sync.dma_start(
                    out=sktab[rows, :].rearrange("(j p) w -> p j w", p=128),
                    in_=sbig[:])
    nc.finalize()
    return nc


def _build_conv(NU, NTAB, H, OUTW, QR, skdirect, NLO, hsplit, unified=False):
    """H heads of 64. kv row = [k(64H)|v(64H)]. NLO: fixed lo-range idx
    count per unit (mult of 128); hi = UE-NLO. hsplit: M' heads 0..hsplit-1
    on DVE, rest on Pool."""
    DH = 64 * H
    QW = max(DH, 128)              # q-table row (>=256B for dma_gather)
    KW = 128 * H
    SW = 65 * H
    SKW = DH
    SLOC = NU * DPB
    SB = SLOC // 512
    nc = bacc.Bacc(dynamic_dma_scratch_size=1 << 17)
    kvf = nc.declare_dram_parameter("kvf", [NTAB, KW], BF16, isOutput=False)
    qtab = None if unified else \
        nc.declare_dram_parameter("qtab", [QR, QW], BF16, isOutput=False)
    sktab = nc.declare_dram_parameter("sktab", [QR, SKW], BF16, isOutput=False)
    midx = nc.declare_dram_parameter("midx", [NU, 128, 136], I16, isOutput=False)
    WO = nc.declare_dram_parameter("WO", [SKW + 1, OUTW], BF16, isOutput=False)
    outt = nc.declare_dram_parameter("outt", [SLOC, OUTW], BF16, isOutput=True)
    stage = nc.dram_tensor("stage", [SLOC, SW], BF16)
    if not skdirect:
        stn = nc.declare_dram_parameter("stn", [SB, 128, 32], I16, isOutput=False)

    with tile.TileContext(nc) as tc:
        with tc.tile_pool(name="one", bufs=1) as one:
            iota = one.tile([128, 64], I16)
            nc.gpsimd.iota(iota[:], pattern=[[1, 64]], base=0,
                           channel_multiplier=0)
            onesrow = one.tile([1, 128], BF16)
            nc.vector.memset(onesrow[:], 1.0)
            onescol = one.tile([128, 1], BF16)
            nc.vector.memset(onescol[:], 1.0)
            ident = one.tile([128, 128], BF16)
            make_identity(nc, ident[:])
            nkw = max(1, SKW // 128)
            kww = SKW // nkw
            WOt = [one.tile([kww, OUTW], BF16, name=f"wo{k}")
                   for k in range(nkw)]
            for k in range(nkw):
                nc.sync.dma_start(out=WOt[k][:], in_=WO[k*kww:(k+1)*kww, :])
            WOb = one.tile([1, OUTW], BF16)
            nc.sync.dma_start(out=WOb[:], in_=WO[SKW:SKW+1, :])

            with tc.tile_pool(name="sb", bufs=3) as sb, \
                 tc.tile_pool(name="sm", bufs=3) as sm, \
                 tc.tile_pool(name="ps", bufs=2, space="PSUM") as ps:
                nlo, nhi = NLO, UE - NLO
                nloc = nlo // 128
                if unified:
                    def kvcol(t):
                        return t if t < nloc else 8 + (t - nloc)
                    def qcol(t):
                        return nloc + t if t < 8 - nloc else 8 + t
                else:
                    kvcol = qcol = None
                for u in range(NU):
                    git = sb.tile([128, 136], I16, tag="git")
                    nc.sync.dma_start(out=git[:], in_=midx[u])
                    slt = sb.tile([128, 8], F32, tag="slt")
                    nc.gpsimd.tensor_copy(slt[:], git[:, 128:136])
                    if unified:
                        gall = sb.tile([128, 16, KW], BF16, tag="kvg")
                        nc.gpsimd.dma_gather(
                            out_ap=gall[:, 0:8, :], in_ap=kvf[0:RSPLIT, :],
                            idxs_ap=git[:, 0:64], num_idxs=UE,
                            num_idxs_reg=UE, elem_size=KW)
                        nc.gpsimd.dma_gather(
                            out_ap=gall[:, 8:16, :], in_ap=kvf[RSPLIT:NTAB, :],
                            idxs_ap=git[:, 64:128], num_idxs=UE,
                            num_idxs_reg=UE, elem_size=KW)
                        kvs = [gall[:, kvcol(t), :] for t in range(TSUB)]
                        qs_ = [gall[:, qcol(t), :] for t in range(TSUB)]
                    else:
                        kvg = sb.tile([128, TSUB, KW], BF16, tag="kvg")
                        nc.gpsimd.dma_gather(
                            out_ap=kvg[:, 0:nloc, :], in_ap=kvf[0:RSPLIT, :],
                            idxs_ap=git[:, 0:nlo // 16], num_idxs=nlo,
                            num_idxs_reg=nlo, elem_size=KW)
                        nc.gpsimd.dma_gather(
                            out_ap=kvg[:, nloc:TSUB, :],
                            in_ap=kvf[RSPLIT:NTAB, :],
                            idxs_ap=git[:, nlo // 16:64],
                            num_idxs=nhi, num_idxs_reg=nhi, elem_size=KW)
                        qg = sb.tile([128, TSUB, QW], BF16, tag="qg")
                        nc.gpsimd.dma_gather(
                            out_ap=qg[:], in_ap=qtab[:], idxs_ap=git[:, 64:128],
                            num_idxs=UE, num_idxs_reg=UE, elem_size=QW)
                        kvs = [kvg[:, t, :] for t in range(TSUB)]
                        qs_ = [qg[:, t, :] for t in range(TSUB)]
                    agg = ps.tile([64, H, 512], F32, tag="agg")
                    for t in range(TSUB):
                        alpha = sm.tile([128, H], F32, tag=f"al{t % 2}")
                        sc = sm.tile([128, H, 64], BF16, tag=f"sc{t % 2}")
                        for h in range(H):
                            nc.vector.scalar_tensor_tensor(
                                out=sc[:, h, :], in0=qs_[t][:, 64*h:64*h+64],
                                scalar=1.0, in1=kvs[t][:, 64*h:64*h+64],
                                op0=mybir.AluOpType.mult,
                                op1=mybir.AluOpType.mult,
                                accum_out=alpha[:, h:h+1])
                        expv = sm.tile([128, H], F32, tag=f"ex{t % 2}")
                        nc.scalar.activation(expv[:], alpha[:],
                                             mybir.ActivationFunctionType.Exp)
                        mp = sm.tile([128, H, 64], BF16, tag=f"mp{t % 2}")
                        for h in range(H):
                            hv = h + 0.5 * (t % 2)
                            eng = nc.vector if hv < hsplit else nc.gpsimd
                            eng.tensor_scalar(
                                out=mp[:, h, :], in0=iota[:],
                                scalar1=slt[:, t:t+1], scalar2=expv[:, h:h+1],
                                op0=mybir.AluOpType.is_equal,
                                op1=mybir.AluOpType.mult)
                        for h in range(H):
                            nc.tensor.matmul(
                                out=agg[:, h, 0:64], lhsT=mp[:, h, :],
                                rhs=kvs[t][:, DH+64*h:DH+64*h+64],
                                start=(t == 0), stop=False,
                                skip_group_check=True)
                            nc.tensor.matmul(
                                out=agg[:, h, 64:65], lhsT=mp[:, h, :],
                                rhs=onescol[:],
                                start=False, stop=(t == TSUB - 1),
                                skip_group_check=True)
                    aggs = sm.tile([64, SW], BF16, tag="aggs")
                    nc.scalar.activation(
                        aggs[:].rearrange("p (h w) -> p h w", h=H),
                        agg[:, :, 0:65], mybir.ActivationFunctionType.Copy)
                    nc.sync.dma_start(out=stage[u * DPB:(u + 1) * DPB, :],
                                      in_=aggs[:])

            with tc.tile_pool(name="sb2", bufs=3) as sb, \
                 tc.tile_pool(name="ps2", bufs=2, space="PSUM") as ps:
                nk = max(1, SKW // 128)
                kw = SKW // nk
                for i in range(SB):
                    r0 = i * 512
                    stg = sb.tile([128, 4, SW], BF16, tag="stg")
                    nc.sync.dma_start(
                        out=stg[:],
                        in_=stage[r0:r0 + 512, :].rearrange(
                            "(c p) w -> p c w", p=128))
                    skt = sb.tile([128, 4, SKW], BF16, tag="skt")
                    if skdirect:
                        nc.sync.dma_start(
                            out=skt[:],
                            in_=sktab[r0:r0 + 512, :].rearrange(
                                "(c p) w -> p c w", p=128))
                    else:
                        sti = sb.tile([128, 32], I16, tag="sti")
                        nc.sync.dma_start(out=sti[:], in_=stn[i])
                        nc.gpsimd.dma_gather(
                            out_ap=skt[:], in_ap=sktab[:], idxs_ap=sti[:],
                            num_idxs=512, num_idxs_reg=512, elem_size=SKW)
                    den = sb.tile([128, 4, H], F32, tag="den")
                    nc.vector.tensor_scalar_max(den[:], stg[:, :, 64::65], 1e-30)
                    rs = sb.tile([128, 4, H], F32, tag="rs")
                    nc.vector.reciprocal(rs[:], den[:])
                    htl = sb.tile([128, 4, SKW], BF16, tag="htl")
                    for c in range(4):
                        for h in range(H):
                            nc.vector.scalar_tensor_tensor(
                                out=htl[:, c, 64*h:64*h+64],
                                in0=stg[:, c, 65*h:65*h+64],
                                scalar=rs[:, c, h:h+1],
                                in1=skt[:, c, 64*h:64*h+64],
                                op0=mybir.AluOpType.mult,
                                op1=mybir.AluOpType.add)
                    relu = sb.tile([128, 4, SKW], BF16, tag="relu")
                    nc.scalar.activation(relu[:], htl[:],
                                         mybir.ActivationFunctionType.Relu)
                    oo = sb.tile([128, 4, OUTW], BF16, tag="oo")
                    for c in range(4):
                        op = ps.tile([128, OUTW], F32, tag="op")
                        nc.tensor.matmul(out=op[:], lhsT=onesrow[:],
                                         rhs=WOb[:], start=True, stop=False,
                                         skip_group_check=True)
                        for k in range(nk):
                            tp = ps.tile([kw, 128], BF16, tag=f"tp{k}")
                            nc.tensor.transpose(
                                out=tp[:], in_=relu[:, c, k*kw:(k+1)*kw],
                                identity=ident[:])
                            tps = sb.tile([kw, 128], BF16, tag=f"tps{k}")
                            nc.scalar.activation(
                                tps[:], tp[:], mybir.ActivationFunctionType.Copy)
                            nc.tensor.matmul(out=op[:], lhsT=tps[:],
                                             rhs=WOt[k][:], start=False,
                                             stop=(k == nk - 1),
                                             skip_group_check=True)
                        nc.vector.tensor_copy(oo[:, c, :], op[:])
                    nc.sync.dma_start(
                        out=outt[r0:r0 + 512, :].rearrange(
                            "(c p) w -> p c w", p=128),
                        in_=oo[:])
    nc.finalize()
    return nc


